# revision 24
# baseline (speedup 1.0000x reference)
# Multi-head attention (BS=2, SL=2048, D=1024, NH=16) on 8 NeuronCores.
#
# Sharding: batch (2) x query-range (4): core c owns batch c//4 and query rows
# [512*(c%4), 512*(c%4+1)). Each core computes the k/v projections for the
# full sequence of its batch (replicated within the batch group -- this
# trades extra PE work for zero inter-core communication), the q projection
# for its own rows, all 16 heads of attention for its rows, and a complete
# 512-row slice of the output projection. The full output is a pure
# concatenation of the 8 per-core slices.
#
# Layout tricks:
#  - q/k kept transposed ([feat, seq], head-dim on partitions) so scores^T
#    comes out with k-position on partitions and the softmax reduction rides
#    the ctx matmul (ones column in v) instead of a cross-partition reduce.
#  - HD=64 means scores matmuls only use half the PE contraction rows, so
#    head PAIRS run concurrently in 64x128 row-tiled mode (tile_position).
#  - exp runs on ScalarE straight out of PSUM in [128,1024] tiles (two heads
#    merged per instruction to amortize the ~220-cycle ACT bubble).
#
# Self-contained: hardcodes shapes; host preps shards (transpose/cast/slice).

import functools

import numpy as np
import ml_dtypes

import concourse.bass as bass
import concourse.mybir as mybir
import concourse.tile as tile
from concourse import bacc
from concourse.bass_utils import run_bass_kernel_spmd

BS, SL, D, NH, HD = 2, 2048, 1024, 16, 64
SCALE = D ** -0.5  # reference scales q by full model dim
NCORES = 8
GROUP = 4                 # cores per batch
QB = SL // GROUP          # q-rows per core = 512
NPAIR = NH // 2           # head pairs = 8

BF16 = mybir.dt.bfloat16
F32 = mybir.dt.float32
NKD = D // 128            # contraction chunks over D = 8
NSEQ = SL // 128          # seq chunks = 16


def _attention_body(nc, tc, debug_taps=False):
    xT = nc.dram_tensor("xT", [D, SL], BF16, kind="ExternalInput")
    xTq = nc.dram_tensor("xTq", [D, QB], BF16, kind="ExternalInput")
    wq = nc.dram_tensor("wq", [D, D], BF16, kind="ExternalInput")
    wk = nc.dram_tensor("wk", [D, D], BF16, kind="ExternalInput")
    wv = nc.dram_tensor("wv", [D, D], BF16, kind="ExternalInput")
    wo = nc.dram_tensor("wo", [D, D], BF16, kind="ExternalInput")
    out = nc.dram_tensor("out", [QB, D], F32, kind="ExternalOutput")
    taps = {}
    if debug_taps:
        for nm, shape in (
            ("d_qT0", [128, QB]), ("d_kT0", [128, SL]), ("d_v0", [128, NH * (HD + 1)]),
            ("d_pt0", [128, 1024]), ("d_pt15", [128, 1024]),
            ("d_ctxT0", [128, QB]),
        ):
            taps[nm] = nc.dram_tensor(nm, shape, BF16, kind="ExternalOutput")

    Exp = mybir.ActivationFunctionType.Exp

    with (
        tc.tile_pool(name="x", bufs=1) as xpool,
        tc.tile_pool(name="w", bufs=1) as wpool,
        tc.tile_pool(name="qk", bufs=1) as qkpool,
        tc.tile_pool(name="vx", bufs=1) as vpool,
        tc.tile_pool(name="pt", bufs=8) as ptpool,
        tc.tile_pool(name="ctx", bufs=1) as ctxpool,
        tc.tile_pool(name="sm", bufs=3) as smpool,
        tc.tile_pool(name="oproj", bufs=2) as opool,
        # PSUM budget (8 banks): mm 2x[128,512] + scores 2x[128,1024] + ctx 2x[65,512]
        tc.tile_pool(name="ps_mm", bufs=2, space="PSUM") as ps_mm,
        tc.tile_pool(name="ps_s", bufs=2, space="PSUM") as ps_s,
        tc.tile_pool(name="ps_c", bufs=2, space="PSUM") as ps_c,
    ):
        qT_sb, kT_sb, v_sb = [], [], []
        # -- qT [D qfeat, QB] transposed projection (scoped wq + xTq) --
        # chunk p holds head pair (2p, 2p+1): head 2p in partitions 0-63,
        # head 2p+1 in partitions 64-127. DMA emission order = need order:
        # wq/xTq (first matmuls), then xT, wk (kT/v projections).
        with tc.tile_pool(name="pwq", bufs=1) as pw:
            wq_sb, xTq_sb = [], []
            for i in range(NKD):
                t = pw.tile([128, D], BF16, tag=f"wq{i}", name=f"wq{i}")
                nc.sync.dma_start(t[:], wq[i * 128:(i + 1) * 128, :])
                wq_sb.append(t)
                tq = pw.tile([128, QB], BF16, tag=f"xTq{i}", name=f"xTq{i}")
                nc.sync.dma_start(tq[:], xTq[i * 128:(i + 1) * 128, :])
                xTq_sb.append(tq)
            xT_sb, wk_sb = [], []
            for i in range(NKD):
                t = xpool.tile([128, SL], BF16, tag=f"xT{i}", name=f"xT{i}")
                nc.sync.dma_start(t[:], xT[i * 128:(i + 1) * 128, :])
                xT_sb.append(t)
                t = wpool.tile([128, D], BF16, tag=f"wk{i}", name=f"wk{i}")
                nc.sync.dma_start(t[:], wk[i * 128:(i + 1) * 128, :])
                wk_sb.append(t)
            for p in range(NPAIR):
                t = qkpool.tile([128, QB], BF16, tag=f"qT{p}", name=f"qT{p}")
                ps = ps_mm.tile([128, 512], F32, tag="mm512")
                for kk in range(NKD):
                    nc.tensor.matmul(
                        ps[:],
                        lhsT=wq_sb[kk][:, p * 128:(p + 1) * 128],
                        rhs=xTq_sb[kk][:],
                        start=(kk == 0),
                        stop=(kk == NKD - 1),
                    )
                nc.vector.tensor_copy(t[:], ps[:])
                if debug_taps and p == 0:
                    nc.sync.dma_start(taps["d_qT0"][:, :], t[:])
                qT_sb.append(t)
        # -- v projection, natural layout, with leading ones column --
        # v_ext[m] is [128 seq, NH*(HD+1)]: head h at cols h*65+1..h*65+64,
        # col h*65 = ones (softmax denominator lands in ctx matmul row 0,
        # i.e. partition 0, where partition_broadcast can source it).
        with tc.tile_pool(name="pwv", bufs=1) as pw:
            wv_sb = []
            for i in range(NKD):
                t = pw.tile([128, D], BF16, tag=f"wv{i}", name=f"wv{i}")
                nc.sync.dma_start(t[:], wv[i * 128:(i + 1) * 128, :])
                wv_sb.append(t)
            for m in range(NSEQ):
                t = vpool.tile(
                    [128, NH * (HD + 1)], BF16, tag=f"v{m}", name=f"v{m}"
                )
                for n2 in range(2):
                    ps = ps_mm.tile([128, 512], F32, tag="mm512")
                    for kk in range(NKD):
                        nc.tensor.matmul(
                            ps[:],
                            lhsT=xT_sb[kk][:, m * 128:(m + 1) * 128],
                            rhs=wv_sb[kk][:, n2 * 512:(n2 + 1) * 512],
                            start=(kk == 0),
                            stop=(kk == NKD - 1),
                        )
                    dst = t[:].rearrange("p (h c) -> p h c", c=HD + 1)[
                        :, n2 * 8:(n2 + 1) * 8, 1:1 + HD
                    ]
                    src = ps[:].rearrange("p (h c) -> p h c", c=HD)
                    nc.vector.tensor_copy(dst, src)
                nc.vector.memset(
                    t[:].rearrange("p (h c) -> p h c", c=HD + 1)[:, :, 0:1],
                    1.0,
                )
                if debug_taps and m == 0:
                    nc.sync.dma_start(taps["d_v0"][:, :], t[:])
                v_sb.append(t)

        # w_out tiles loaded after `pin` closes to keep the projection-phase
        # SBUF peak down; pair p's two heads stacked on the partition axis.
        wo_sb = []
        for p in range(NPAIR):
            t = wpool.tile([128, D], BF16, tag=f"wo{p}", name=f"wo{p}")
            nc.sync.dma_start(t[:], wo[p * 128:(p + 1) * 128, :])
            wo_sb.append(t)

        # ---- attention per head pair ----
        # ctxT pair tile [128, QB]: head 2p rows 0-63, head 2p+1 rows 64-127
        # (head B lands there via a small SBUF->SBUF DMA since DVE cannot
        # shift partitions).
        ctxT_sb = []
        for p in range(NPAIR):
            ctxT_sb.append(
                ctxpool.tile([128, QB], BF16, tag=f"ctxT{p}", name=f"ctxT{p}")
            )
        def emit_normalize(p, cps):
            # normalize rows 1..64 by row 0 (ones-row dot = exp row-sum);
            # emitted AFTER the next pair's kT copies so the cheap approx
            # reciprocal chain does not head-of-line block the DVE FIFO.
            for hh in range(2):
                recip = smpool.tile(
                    [1, 512], F32, tag="recip", name=f"recip{p}_{hh}"
                )
                nc.vector.reciprocal_approx_fast(recip[:], cps[hh][0:1, :])
                rbc = smpool.tile([65, 512], F32, tag="rbc", name=f"rbc{p}_{hh}")
                nc.gpsimd.partition_broadcast(rbc[:], recip[:])
                stage = smpool.tile(
                    [65, 512], BF16, tag="stage", name=f"stage{p}_{hh}"
                )
                nc.vector.tensor_tensor(
                    stage[0:64, :], cps[hh][0:64, :], rbc[0:64, :],
                    mybir.AluOpType.mult,
                )
                nc.vector.tensor_tensor(
                    stage[64:65, :], cps[hh][64:65, :], rbc[64:65, :],
                    mybir.AluOpType.mult,
                )
                nc.sync.dma_start(
                    ctxT_sb[p][hh * 64:(hh + 1) * 64, :], stage[1:65, :]
                )

        prev = None
        for p in range(NPAIR):
            # kT projection for this pair, interleaved so it fills PE gaps
            # while the previous pair's (ACT-bound) softmax runs.
            t = qkpool.tile([128, SL], BF16, tag=f"kT{p}", name=f"kT{p}")
            for n in range(SL // 512):
                ps = ps_mm.tile([128, 512], F32, tag="mm512")
                for kk in range(NKD):
                    nc.tensor.matmul(
                        ps[:],
                        lhsT=wk_sb[kk][:, p * 128:(p + 1) * 128],
                        rhs=xT_sb[kk][:, n * 512:(n + 1) * 512],
                        start=(kk == 0),
                        stop=(kk == NKD - 1),
                    )
                nc.vector.tensor_copy(t[:, n * 512:(n + 1) * 512], ps[:])
            if debug_taps and p == 0:
                nc.sync.dma_start(taps["d_kT0"][:, :], t[:])
            kT_sb.append(t)

            if prev is not None:
                emit_normalize(p - 1, prev)

            cps = [ps_c.tile([65, 512], F32, tag="ctx", name=f"cps{p}_{hh}")
                   for hh in range(2)]
            pts = {}

            def emit_ctx(m):
                for hh in range(2):
                    h = 2 * p + hh
                    nc.tensor.matmul(
                        cps[hh][:],
                        lhsT=v_sb[m][:, h * 65:(h + 1) * 65],
                        rhs=pts[m][:, hh * 512:(hh + 1) * 512],
                        start=(m == 0),
                        stop=(m == NSEQ - 1),
                    )

            for m in range(NSEQ):
                ps = ps_s.tile([128, 1024], F32, tag="scores")
                nc.tensor.matmul(
                    ps[:, 0:512],
                    lhsT=kT_sb[p][0:64, m * 128:(m + 1) * 128],
                    rhs=qT_sb[p][0:64, :],
                    start=True, stop=True,
                    tile_position=(0, 0),
                )
                nc.tensor.matmul(
                    ps[:, 512:1024],
                    lhsT=kT_sb[p][64:128, m * 128:(m + 1) * 128],
                    rhs=qT_sb[p][64:128, :],
                    start=True, stop=True,
                    tile_position=(64, 0),
                )
                pt = ptpool.tile([128, 1024], BF16, tag="pt")
                nc.scalar.activation(pt[:], ps[:], Exp)
                if debug_taps and p == 0 and m in (0, NSEQ - 1):
                    nc.sync.dma_start(taps[f"d_pt{m}"][:, :], pt[:])
                pts[m] = pt
                # ctx lags scores by 2 chunks so the PE never waits on exp
                if m >= 2:
                    emit_ctx(m - 2)
                    del pts[m - 2]
            emit_ctx(NSEQ - 2)
            emit_ctx(NSEQ - 1)
            prev = cps
        emit_normalize(NPAIR - 1, prev)

        # ---- output projection: out[QB, D] = ctx[QB, D] @ w_out ----
        for n in range(D // 512):
            for mq in range(QB // 128):
                ps = ps_mm.tile([128, 512], F32, tag="mm512")
                for p in range(NPAIR):
                    nc.tensor.matmul(
                        ps[:],
                        lhsT=ctxT_sb[p][:, mq * 128:(mq + 1) * 128],
                        rhs=wo_sb[p][:, n * 512:(n + 1) * 512],
                        start=(p == 0),
                        stop=(p == NPAIR - 1),
                    )
                o = opool.tile([128, 512], F32, tag="osb")
                nc.vector.tensor_copy(o[:], ps[:])
                nc.sync.dma_start(
                    out[mq * 128:(mq + 1) * 128, n * 512:(n + 1) * 512], o[:]
                )


@functools.lru_cache(maxsize=2)
def _build(debug_taps=False):
    nc = bacc.Bacc(
        "TRN2",
        target_bir_lowering=False,
        debug=False,
        enable_asserts=True,
        num_devices=NCORES,
    )
    with tile.TileContext(nc) as tc:
        _attention_body(nc, tc, debug_taps)
    nc.compile()
    return nc


def make_in_maps(input_sequence, w_qkv, w_out):
    bf16 = ml_dtypes.bfloat16
    x = np.asarray(input_sequence, dtype=np.float32)
    w_qkv = np.asarray(w_qkv, dtype=np.float32)
    w_out = np.asarray(w_out, dtype=np.float32)

    xT = [np.ascontiguousarray(x[b].T).astype(bf16) for b in range(BS)]
    wq_ = np.ascontiguousarray(w_qkv[:, :D] * SCALE).astype(bf16)
    wk_ = np.ascontiguousarray(w_qkv[:, D:2 * D]).astype(bf16)
    wv_ = np.ascontiguousarray(w_qkv[:, 2 * D:]).astype(bf16)
    wo_ = np.ascontiguousarray(w_out).astype(bf16)
    in_maps = []
    for c in range(NCORES):
        b, r = divmod(c, GROUP)
        in_maps.append({
            "xT": xT[b],
            "xTq": np.ascontiguousarray(xT[b][:, r * QB:(r + 1) * QB]),
            "wq": wq_, "wk": wk_, "wv": wv_, "wo": wo_,
        })
    return in_maps


def assemble_output(results):
    out = np.empty((BS, SL, D), dtype=np.float32)
    for c in range(NCORES):
        b, r = divmod(c, GROUP)
        out[b, r * QB:(r + 1) * QB, :] = results[c]["out"]
    return out


def kernel(input_sequence, w_qkv, w_out, _trace=False, _results=[None]):
    nc = _build()
    in_maps = make_in_maps(input_sequence, w_qkv, w_out)
    res = run_bass_kernel_spmd(
        nc, in_maps, core_ids=list(range(NCORES)), trace=_trace
    )
    _results[0] = res
    return assemble_output(res.results)
